# revision 10
# baseline (speedup 1.0000x reference)
"""Trainium2 Bass kernel for nn_Criterion_33947421508242.

Multi-level trajectory-prediction loss (Laplace NLL + mode posterior +
bivariate-Gaussian entropy + KL + FDE/ADE), pure data parallel over the
batch axis across 8 NeuronCores.

Hardcoded problem shape: B=2048, N=8, K=6, T=60, 3 levels, f32.
Each core processes B/8=256 batch rows -> 2048 (b,n) samples -> 16
partition tiles of 128 samples.  Per-sample/level/k statistics are
reduced on-chip; the final (linear) mean over batch is done on host.
"""

import math

import numpy as np

# problem constants (hardcoded per task instructions)
B, N, K, T = 2048, 8, 6, 60
N_CORES = 8
NL = 3  # levels
ENTROPY_WEIGHT = 40.0
KL_WEIGHT = 20.0
LOG_2PI_E = float(1.0 + np.log(2.0 * np.pi))
LN4 = float(np.log(4.0))

B_SH = B // N_CORES          # batch rows per core
S = B_SH * N                 # (b,n) samples per core = 2048
P = 128                      # partitions
NT = S // P                  # 16 tiles per core
KT = K * T                   # 360
CH = NL * KT                 # 1080 elems per channel per tile
GD = NL * K                  # 18 groups (level,k) per tile
FREE5 = NL * KT * 5          # 5400 f32 per partition (traj)


def _emit(nc, tc, tile_mod, mybir, n_tiles, t_dram, gt_dram, pr_dram, out_dram):
    import concourse.bass as bass
    from contextlib import ExitStack

    f32 = mybir.dt.float32
    Alu = mybir.AluOpType
    Act = mybir.ActivationFunctionType
    Ax = mybir.AxisListType

    ctx = ExitStack()
    with ctx:
        io = ctx.enter_context(tc.tile_pool(name="io", bufs=2))
        wk = ctx.enter_context(tc.tile_pool(name="wk", bufs=1))
        st = ctx.enter_context(tc.tile_pool(name="st", bufs=1))

        bf16 = mybir.dt.bfloat16
        GDT = n_tiles * GD  # stats free size
        stat_A = st.tile([P, GDT], f32, tag="stA")    # sum_t m*(L+s)
        stat_E = st.tile([P, GDT], f32, tag="stE")    # sum_t (L + 0.5*G12)
        stat_D = st.tile([P, GDT], f32, tag="stD")    # sum_t m*dist
        stat_F = st.tile([P, GDT], f32, tag="stF")    # fde = (m*dist)[T-1]
        stat_P = st.tile([P, GDT], f32, tag="stP")    # probs
        stat_NM = st.tile([P, n_tiles], f32, tag="stNM")  # nmask

        def bcast(ap, rep):
            """insert [0, rep] after partition dim of a 2d free AP"""
            return bass.AP(tensor=ap.tensor, offset=ap.offset,
                           ap=[ap.ap[0], [0, rep], *ap.ap[1:]])

        for ts in range(n_tiles):
            sl = slice(ts * P, (ts + 1) * P)
            gsl = slice(ts * GD, (ts + 1) * GD)

            traj = io.tile([P, FREE5], f32, tag="traj")
            for l in range(NL):
                nc.sync.dma_start(
                    out=traj[:, l * KT * 5:(l + 1) * KT * 5],
                    in_=t_dram[l][sl, :])
            gt = io.tile([P, T * 3], f32, tag="gt")
            nc.sync.dma_start(out=gt[:, :], in_=gt_dram[sl, :])
            nc.sync.dma_start(out=stat_P[:, gsl], in_=pr_dram[sl, :])

            tv = traj[:, :].rearrange("p (n c) -> p n c", c=5)
            mux, muy, sx, sy, rho = (tv[:, :, c] for c in range(5))
            gv = gt[:, :].rearrange("p (t c) -> p t c", c=3)
            x_r = bcast(gv[:, :, 0], GD)
            y_r = bcast(gv[:, :, 1], GD)

            # mask cast to contiguous bf16 once per tile for 2x TT mode
            mb = wk.tile([P, T], bf16, tag="mb")
            nc.vector.tensor_copy(out=mb[:, :], in_=gv[:, :, 2])
            m_rb = bcast(mb[:, :], GD)

            dx = wk.tile([P, CH], bf16, tag="dx")
            dy = wk.tile([P, CH], bf16, tag="dy")
            nc.vector.tensor_sub(out=dx[:, :], in0=x_r, in1=mux)
            nc.vector.tensor_sub(out=dy[:, :], in0=y_r, in1=muy)

            # per-channel ln and reciprocal on ACT:
            #   lnp = ln(sx*sy) = ln sx + ln sy;  1/sx = exp(-ln sx)
            lsx = wk.tile([P, CH], bf16, tag="lsx")
            lsy = wk.tile([P, CH], bf16, tag="lsy")
            nc.scalar.activation(out=lsx[:, :], in_=sx, func=Act.Ln)
            nc.scalar.activation(out=lsy[:, :], in_=sy, func=Act.Ln)
            sxi = wk.tile([P, CH], bf16, tag="sxi")
            syi = wk.tile([P, CH], bf16, tag="syi")
            nc.scalar.activation(out=sxi[:, :], in_=lsx[:, :], func=Act.Exp,
                                 scale=-1.0)
            nc.scalar.activation(out=syi[:, :], in_=lsy[:, :], func=Act.Exp,
                                 scale=-1.0)
            lnp = wk.tile([P, CH], bf16, tag="lnp")
            nc.vector.tensor_add(out=lnp[:, :], in0=lsx[:, :], in1=lsy[:, :])

            adx = wk.tile([P, CH], bf16, tag="adx")
            ady = wk.tile([P, CH], bf16, tag="ady")
            nc.scalar.activation(out=adx[:, :], in_=dx[:, :], func=Act.Abs)
            nc.scalar.activation(out=ady[:, :], in_=dy[:, :], func=Act.Abs)
            w1 = wk.tile([P, CH], bf16, tag="w1")
            w2 = wk.tile([P, CH], bf16, tag="w2")
            nc.vector.tensor_mul(out=w1[:, :], in0=adx[:, :], in1=sxi[:, :])
            nc.vector.tensor_mul(out=w2[:, :], in0=ady[:, :], in1=syi[:, :])
            ss = wk.tile([P, CH], bf16, tag="ss")
            nc.vector.tensor_add(out=ss[:, :], in0=w1[:, :], in1=w2[:, :])
            uu = wk.tile([P, CH], bf16, tag="uu")
            nc.vector.tensor_add(out=uu[:, :], in0=ss[:, :], in1=lnp[:, :])
            um = wk.tile([P, CH], bf16, tag="um")
            nc.gpsimd.tensor_mul(out=um[:, :], in0=uu[:, :], in1=m_rb)
            umv = um[:, :].rearrange("p (g t) -> p g t", t=T)
            nc.vector.reduce_sum(out=stat_A[:, gsl], in_=umv, axis=Ax.X)

            # entropy pieces: integrand L + 0.5*(ln(1-rho)+ln(1+rho))
            g1 = wk.tile([P, CH], bf16, tag="g1")
            g2 = wk.tile([P, CH], bf16, tag="g2")
            nc.scalar.activation(out=g1[:, :], in_=rho, func=Act.Ln,
                                 scale=-1.0, bias=1.0)
            nc.scalar.activation(out=g2[:, :], in_=rho, func=Act.Ln,
                                 scale=1.0, bias=1.0)
            g12 = wk.tile([P, CH], bf16, tag="g12")
            nc.gpsimd.tensor_add(out=g12[:, :], in0=g1[:, :], in1=g2[:, :])
            eint = wk.tile([P, CH], bf16, tag="eint")
            nc.vector.scalar_tensor_tensor(out=eint[:, :], in0=g12[:, :],
                                           scalar=0.5, in1=lnp[:, :],
                                           op0=Alu.mult, op1=Alu.add)
            ev = eint[:, :].rearrange("p (g t) -> p g t", t=T)
            nc.vector.reduce_sum(out=stat_E[:, gsl], in_=ev, axis=Ax.X)

            # fde / ade distances
            dx2 = wk.tile([P, CH], bf16, tag="dx2")
            dy2 = wk.tile([P, CH], bf16, tag="dy2")
            nc.gpsimd.tensor_mul(out=dx2[:, :], in0=dx[:, :], in1=dx[:, :])
            nc.vector.tensor_mul(out=dy2[:, :], in0=dy[:, :], in1=dy[:, :])
            dd = wk.tile([P, CH], bf16, tag="dd")
            nc.vector.tensor_add(out=dd[:, :], in0=dx2[:, :], in1=dy2[:, :])
            lnd = wk.tile([P, CH], bf16, tag="lnd")
            nc.scalar.activation(out=lnd[:, :], in_=dd[:, :], func=Act.Ln)
            dist = wk.tile([P, CH], bf16, tag="dist")
            nc.scalar.activation(out=dist[:, :], in_=lnd[:, :], func=Act.Exp,
                                 scale=0.5)
            dm = wk.tile([P, CH], bf16, tag="dm")
            nc.gpsimd.tensor_mul(out=dm[:, :], in0=dist[:, :], in1=m_rb)
            dmv = dm[:, :].rearrange("p (g t) -> p g t", t=T)
            nc.vector.reduce_sum(out=stat_D[:, gsl], in_=dmv, axis=Ax.X)
            nc.vector.tensor_copy(out=stat_F[:, gsl], in_=dmv[:, :, T - 1])

            # nmask = sum_t m
            nc.vector.reduce_sum(out=stat_NM[:, ts:ts + 1], in_=gv[:, :, 2],
                                 axis=Ax.X)

        # ---------- finishing phase over [P, GDT] ----------
        NG = n_tiles * NL  # per-(tile,level) groups

        def gview(t):  # [P, NG, K]
            return t[:, :].rearrange("p (g k) -> p g k", k=K)

        def kb(t):  # broadcast [P, NG] -> (NG, K)
            a = t[:, :]
            return bass.AP(tensor=a.tensor, offset=a.offset,
                           ap=[a.ap[0], a.ap[1], [0, K]])

        fin = ctx.enter_context(tc.tile_pool(name="fin", bufs=1))

        nm4 = fin.tile([P, n_tiles], f32, tag="nm4")
        nc.vector.tensor_scalar_mul(out=nm4[:, :], in0=stat_NM[:, :], scalar1=LN4)
        nm4b = bass.AP(tensor=nm4[:, :].tensor, offset=nm4[:, :].offset,
                       ap=[nm4[:, :].ap[0], nm4[:, :].ap[1], [0, GD]])

        lpr = fin.tile([P, GDT], f32, tag="lpr")
        nc.scalar.activation(out=lpr[:, :], in_=stat_P[:, :], func=Act.Ln)
        nll = fin.tile([P, GDT], f32, tag="nll")
        nc.vector.tensor_add(out=nll[:, :], in0=stat_A[:, :], in1=nm4b)
        logit = fin.tile([P, GDT], f32, tag="logit")
        nc.vector.tensor_sub(out=logit[:, :], in0=lpr[:, :], in1=nll[:, :])
        lmax = fin.tile([P, NG], f32, tag="lmax")
        nc.vector.reduce_max(out=lmax[:, :], in_=gview(logit), axis=Ax.X)
        zz = fin.tile([P, GDT], f32, tag="zz")
        nc.vector.tensor_sub(out=zz[:, :], in0=logit[:, :], in1=kb(lmax))
        ee = fin.tile([P, GDT], f32, tag="ee")
        nc.scalar.activation(out=ee[:, :], in_=zz[:, :], func=Act.Exp)
        sume = fin.tile([P, NG], f32, tag="sume")
        nc.vector.reduce_sum(out=sume[:, :], in_=gview(ee), axis=Ax.X)
        lnse = fin.tile([P, NG], f32, tag="lnse")
        nc.scalar.activation(out=lnse[:, :], in_=sume[:, :], func=Act.Ln)
        lpost = fin.tile([P, GDT], f32, tag="lpost")
        nc.vector.tensor_sub(out=lpost[:, :], in0=zz[:, :], in1=kb(lnse))
        post = fin.tile([P, GDT], f32, tag="post")
        nc.scalar.activation(out=post[:, :], in_=lpost[:, :], func=Act.Exp)
        # faithful to reference: log(post) (-inf at 0) so 0*(-inf)=NaN matches
        lp2 = fin.tile([P, GDT], f32, tag="lp2")
        nc.scalar.activation(out=lp2[:, :], in_=post[:, :], func=Act.Ln)

        t1 = fin.tile([P, GDT], f32, tag="t1")
        nc.vector.tensor_mul(out=t1[:, :], in0=post[:, :], in1=nll[:, :])
        loss1 = fin.tile([P, NG], f32, tag="loss1")
        nc.vector.reduce_sum(out=loss1[:, :], in_=gview(t1), axis=Ax.X)

        kd = fin.tile([P, GDT], f32, tag="kd")
        nc.vector.tensor_sub(out=kd[:, :], in0=lp2[:, :], in1=lpr[:, :])
        t2 = fin.tile([P, GDT], f32, tag="t2")
        nc.vector.tensor_mul(out=t2[:, :], in0=post[:, :], in1=kd[:, :])
        kls = fin.tile([P, NG], f32, tag="kls")
        nc.vector.reduce_sum(out=kls[:, :], in_=gview(t2), axis=Ax.X)

        fa = fin.tile([P, GDT], f32, tag="fa")
        nc.vector.scalar_tensor_tensor(out=fa[:, :], in0=stat_D[:, :],
                                       scalar=1.0 / T, in1=stat_F[:, :],
                                       op0=Alu.mult, op1=Alu.add)
        famin = fin.tile([P, NG], f32, tag="famin")
        nc.vector.tensor_reduce(out=famin[:, :], in_=gview(fa), axis=Ax.X,
                                op=Alu.min)

        entmax = fin.tile([P, NG], f32, tag="entmax")
        nc.vector.reduce_max(out=entmax[:, :], in_=gview(stat_E), axis=Ax.X)

        g1t = fin.tile([P, NG], f32, tag="g1t")
        nc.vector.scalar_tensor_tensor(out=g1t[:, :], in0=entmax[:, :],
                                       scalar=ENTROPY_WEIGHT, in1=loss1[:, :],
                                       op0=Alu.mult, op1=Alu.add)
        g2t = fin.tile([P, NG], f32, tag="g2t")
        nc.vector.scalar_tensor_tensor(out=g2t[:, :], in0=kls[:, :],
                                       scalar=KL_WEIGHT, in1=g1t[:, :],
                                       op0=Alu.mult, op1=Alu.add)
        g3t = fin.tile([P, NG], f32, tag="g3t")
        nc.vector.scalar_tensor_tensor(out=g3t[:, :], in0=famin[:, :],
                                       scalar=100.0, in1=g2t[:, :],
                                       op0=Alu.mult, op1=Alu.add)
        g4t = fin.tile([P, NG], f32, tag="g4t")
        nc.vector.tensor_scalar_add(out=g4t[:, :], in0=g3t[:, :],
                                    scalar1=ENTROPY_WEIGHT * T * LOG_2PI_E)
        nc.sync.dma_start(out=out_dram[:, :], in_=g4t[:, :])


def build_program(n_tiles=NT):
    import concourse.bacc as bacc
    import concourse.tile as tile
    from concourse import mybir

    class _Bacc(bacc.Bacc):
        # All activations used here (Ln, Exp, Abs) live in the
        # natural_log_exp_and_others table set.  The default chooser picks
        # the first set containing each function, thrashing between sets
        # (one ~2.7us ACT_TABLE_LOAD per transition); restricting the
        # candidate list yields a single hoisted load.
        def insert_act_table_loads(self):
            import bass_rust as _bass_rust
            from concourse.hw_specs import get_activation_tables

            tables = [(k, v)
                      for k, v in get_activation_tables(self.m.arch).items()
                      if k == "natural_log_exp_and_others"]
            _bass_rust.insert_act_table_loads(self, tables)

    f32 = mybir.dt.float32
    nc = _Bacc(name="criterion_loss")
    s = n_tiles * P
    t_dram = [nc.dram_tensor(f"traj{l}", [s, KT * 5], f32, kind="ExternalInput")
              for l in range(NL)]
    gt_dram = nc.dram_tensor("gt", [s, T * 3], f32, kind="ExternalInput")
    pr_dram = nc.dram_tensor("probs", [s, GD], f32, kind="ExternalInput")
    out_dram = nc.dram_tensor("out", [P, n_tiles * NL], f32,
                              kind="ExternalOutput")
    with tile.TileContext(nc) as tc:
        _emit(nc, tc, tile, mybir, n_tiles,
              [t.ap() for t in t_dram], gt_dram.ap(), pr_dram.ap(),
              out_dram.ap())
    nc.compile()
    return nc


def make_in_map(traj, probs, data, core, n_tiles=NT):
    """Build one core's input map from full (unsharded) np arrays."""
    s = n_tiles * P
    bsh = s // N
    b0 = core * bsh
    m = {}
    for l in range(NL):
        m[f"traj{l}"] = np.ascontiguousarray(
            traj[l][b0:b0 + bsh].reshape(s, KT * 5))
    m["gt"] = np.ascontiguousarray(data[b0:b0 + bsh].reshape(s, T * 3))
    m["probs"] = np.ascontiguousarray(
        np.concatenate([probs[l][b0:b0 + bsh].reshape(s, K) for l in range(NL)],
                       axis=1))
    return m


def _kernel_numpy_fallback(traj, probs, data):
    """Reference-faithful numpy path for unexpected N_levels (not used for
    the standard shape)."""
    x = data[..., 0]
    y = data[..., 1]
    m = data[..., 2]
    nl = len(traj)
    total = np.float64(0.0)
    for l in range(nl):
        t5 = traj[l]
        mux, muy, bx, by, rho = (t5[..., i] for i in range(5))
        dx = x[:, :, None, :] - mux
        dy = y[:, :, None, :] - muy
        with np.errstate(all="ignore"):
            nll_t = (np.log(2 * bx) + np.abs(dx) / bx
                     + np.log(2 * by) + np.abs(dy) / by)
            nll = (nll_t * m[:, :, None, :]).sum(-1)
            lpu = -nll + np.log(probs[l])
            zmax = lpu.max(-1, keepdims=True)
            e = np.exp(lpu - zmax)
            post = e / e.sum(-1, keepdims=True)
            loss1 = (post * nll).sum(-1)
            ent = (LOG_2PI_E + np.log(bx * by)
                   + 0.5 * np.log1p(-rho * rho)).sum(-1)
            entmax = ent.max(-1)
            kl = (post * (np.log(post) - np.log(probs[l]))).sum(-1)
            dist = np.sqrt(dx * dx + dy * dy)
            fde = dist[..., -1] * m[:, :, None, -1]
            ade = (dist * m[:, :, None, :]).mean(-1)
            famin = (fde + ade).min(-1)
            total += (loss1 + 40.0 * entmax + 20.0 * kl + 100.0 * famin).sum()
    n_ag = traj[0].shape[1]
    return np.float32(total / traj[0].shape[0] / nl * n_ag)


_PROGRAM_CACHE = {}


def kernel(**inputs):
    level_names = sorted(k for k in inputs if k.endswith("_trajectory"))
    nl = len(level_names)
    traj = [np.asarray(inputs[f"level_{l}_trajectory"], dtype=np.float32)
            for l in range(nl)]
    probs = [np.asarray(inputs[f"level_{l}_probability"], dtype=np.float32)
             for l in range(nl)]
    data = np.asarray(inputs["data"], dtype=np.float32)
    n_levels = int(np.asarray(inputs["N_levels"]))

    if (n_levels != 2 or nl != 3 or traj[0].shape != (B, N, K, T, 5)
            or data.shape != (B, N, T, 3)):
        return _kernel_numpy_fallback(traj[:n_levels + 1], probs, data)

    from concourse.bass_utils import run_bass_kernel_spmd

    if NT not in _PROGRAM_CACHE:
        _PROGRAM_CACHE[NT] = build_program(NT)
    nc = _PROGRAM_CACHE[NT]

    in_maps = [make_in_map(traj, probs, data, c) for c in range(N_CORES)]
    res = run_bass_kernel_spmd(nc, in_maps, core_ids=list(range(N_CORES)))
    total = np.float64(0.0)
    for r in res.results:
        total += np.asarray(r["out"], dtype=np.float64).sum()
    n_agents = N
    return np.float32(total / B / NL * n_agents)


if __name__ == "__main__":
    import reference as R  # only when run manually inside /root/problem

    inp = {k: np.asarray(v) if not np.isscalar(v) else v
           for k, v in R.setup_inputs().items()}
    out = kernel(**inp)
    exp = np.asarray(R.reference(**R.setup_inputs()))
    print("kernel:", out, "reference:", exp)
